# revision 26
# baseline (speedup 1.0000x reference)
"""GPT forward (V=32000,S=1024,D=768,L=6,H=12,FF=3072,B=4) on 8 trn2 NeuronCores.

Sharding: DP=4 core-pairs over batch B; TP=2 (Megatron) inside each pair:
  heads 6+6, FF 1536+1536, vocab 16000+16000 for the logits GEMM.
All GEMMs run in bf16 (PE 1 cycle/row vs 4 for fp32); PSUM accumulation is
fp32. Residual h is bf16.

The layer loop is software-pipelined over the two 512-token halves:
  Attn(h1) -> Attn(h2) -> MLP(h1) -> MLP(h2)
with one pairwise bf16 AllReduce per (phase, half) issued at phase end and
consumed one phase later, so every AR (~17us) hides behind ~30us of matmul
work. All layer weights are preloaded into SBUF at layer start (used by
both halves; pool rotation doubles as cross-layer prefetch).

Attention is transposed (sT[k,q]) per (head-pair, key-block) with the kt
loop software-pipelined: PV(kt-1) trails scores(kt) one step so the tensor
queue rarely waits on the scalar-engine Exp. Causal mask is added in PSUM
via a constant-matrix matmul (-240 strictly lower). The softmax denominator
rides as a 65th ones-column of V; the reciprocal is broadcast with two K=1
matmuls per head pair, and the odd head is relocated to partitions 64:128
by a small SBUF->SBUF DMA so out_proj runs with K=128 over packed head
pairs (half the matmuls of the K=64 layout). out_proj is hp-outer in two
m-groups so pair-0/1 matmuls overlap the last pair's normalize chain.

LayerNorm stats come from ones-matmul column sums (squares on the vector
engine), scale/shift broadcast across partitions with K=1 matmuls, applied
as two bf16 vector ops; tiny dependency-gated dummy matmuls ("HAM tickles")
keep the PE activity window non-idle across the LN scalar/vector chain.
out_proj/qk biases are applied via activation-bias (no ones-matmul).
The lm_head streams 32 vocab blocks of 500 with the first 4 blocks split
by token half so the last AllReduce hides behind half-0 logits GEMMs.
"""

import os
import sys

import numpy as np

for _p in ("/opt/trn_rl_repo",):
    if _p not in sys.path:
        sys.path.insert(0, _p)

V, S, D, L, H, FF = 32000, 1024, 768, 6, 12, 3072
B, T = 4, 1024
HD = D // H            # 64
NC_ = 8                # cores
TP = 2
NH = H // TP           # 6 local heads
DQK = NH * HD          # 384
FFSH = FF // TP        # 1536
VSH = V // TP          # 16000
P = 128
KD = D // P            # 6 k-chunks of d_model
KFF = FFSH // P        # 12
NT = T // P            # 8 token chunks
HW = 512               # token half width
VBLK = 500             # vocab free-block
VN = VSH // VBLK       # 32
EPS = 1e-5
VW = 65                # v columns per head incl. ones column
MASKVAL = -240.0

_CACHE = {}


# --------------------------------------------------------------------------
# host-side input preparation (sharding + layout + LN folding)
# --------------------------------------------------------------------------

def _lhsT_layout(Wf, nm, nk):
    """Wf [nm*128 out, nk*128 in] -> [nm, 128(p=in%128), nk, 128(c=out%128)]
    so that sbuf tile[p, k*128+c] = Wf[m*128+c, k*128+p]."""
    return np.ascontiguousarray(
        Wf.reshape(nm, P, nk, P).transpose(0, 3, 2, 1)
    )


def _rhs_layout(Wf, nk, nblk):
    """Wf [nblk out, nk*128 in] -> [128(p), nk, nblk]: tile[p, k, c] = Wf[c, k*128+p]."""
    return np.ascontiguousarray(
        Wf.reshape(nblk, nk, P).transpose(2, 1, 0)
    )


def _bias_layout(b, nm):
    """b [nm*128] -> [128, nm]"""
    return np.ascontiguousarray(b.reshape(nm, P).T)


def prep_rank_weights(r, tok_emb, pos_emb, ln1_w, ln1_b, qkv_w, out_w,
                      ln2_w, ln2_b, up_w, down_w, lnf_w, lnf_b):
    """Weights depend only on the TP rank r (shared across the 4 DP pairs)."""
    import ml_dtypes
    bf16 = ml_dtypes.bfloat16
    f32 = np.float32

    inp = {}
    wqk = np.empty((L, KD, P, KD, P), bf16)
    bqk = np.empty((L, P, KD), f32)
    wv = np.empty((L, P, KD, DQK), bf16)
    wo = np.empty((L, KD, P, 3, P), bf16)
    bo = np.empty((L, P, KD), f32)
    wup = np.empty((L, KFF, P, KD, P), bf16)
    bup = np.empty((L, P, KFF), f32)
    wdn = np.empty((L, KD, P, KFF, P), bf16)

    hsel = slice(r * DQK, (r + 1) * DQK)
    for l in range(L):
        q_raw = qkv_w[l, 0 * D + r * DQK: 0 * D + (r + 1) * DQK]   # [384, 768]
        k_raw = qkv_w[l, 1 * D + r * DQK: 1 * D + (r + 1) * DQK]
        v_raw = qkv_w[l, 2 * D + r * DQK: 2 * D + (r + 1) * DQK]
        qk_raw = np.concatenate([q_raw, k_raw], 0)                 # [768, 768]
        wqk[l] = _lhsT_layout(qk_raw * ln1_w[l][None, :], KD, KD)
        bqk[l] = _bias_layout(qk_raw @ ln1_b[l], KD)
        wv[l] = _rhs_layout(v_raw * ln1_w[l][None, :], KD, DQK)
        bv = v_raw @ ln1_b[l]                                      # [384]
        wo_raw = out_w[l][:, hsel]                                 # [768, 384]
        # halved: the pair AllReduce of (h/2 + Wo/2 y + bo/2) returns h_new/2
        wo[l] = _lhsT_layout(0.5 * wo_raw, KD, 3)
        bo[l] = _bias_layout(0.5 * (wo_raw @ bv), KD)
        up_raw = up_w[l, r * FFSH:(r + 1) * FFSH]                  # [1536, 768]
        wup[l] = _lhsT_layout(up_raw * ln2_w[l][None, :], KFF, KD)
        bup[l] = _bias_layout(up_raw @ ln2_b[l], KFF)
        dn_raw = down_w[l][:, r * FFSH:(r + 1) * FFSH]             # [768, 1536]
        wdn[l] = _lhsT_layout(0.5 * dn_raw, KD, KFF)

    inp["wqk"], inp["bqk"], inp["wv"] = wqk, bqk, wv
    inp["wo"], inp["bo"] = wo, bo
    inp["wup"], inp["bup"], inp["wdn"] = wup, bup, wdn

    te = tok_emb[r * VSH:(r + 1) * VSH].astype(f32) * lnf_w[None, :].astype(f32)
    # [VN, 128, KD, VBLK]: tile[n, p, k, c] = te[n*VBLK + c, k*128 + p]
    inp["temb"] = np.ascontiguousarray(
        te.reshape(VN, VBLK, KD, P).transpose(0, 3, 2, 1)
    ).astype(bf16)
    return inp


def prep_all_inputs(**inputs):
    import ml_dtypes
    bf16 = ml_dtypes.bfloat16
    f32 = np.float32
    args = {k: np.asarray(v) for k, v in inputs.items()}
    for k in args:
        if args[k].dtype in (np.float64,):
            args[k] = args[k].astype(f32)
    idx = args.pop("idx")
    rank_w = [prep_rank_weights(r, **args) for r in range(TP)]

    in_maps = []
    for c in range(NC_):
        b, r = c // TP, c % TP
        inp = dict(rank_w[r])
        h0 = 0.5 * (args["tok_emb"][idx[b]] + args["pos_emb"][:T]).astype(f32).T
        inp["h0"] = np.ascontiguousarray(
            h0.reshape(KD, P, T).transpose(1, 0, 2)).astype(bf16)
        in_maps.append(inp)
    return in_maps


# --------------------------------------------------------------------------
# bass program
# --------------------------------------------------------------------------

def build_program():
    import concourse.bass as bass
    import concourse.mybir as mybir
    import concourse.tile as tile
    from concourse import bacc
    from concourse.masks import make_upper_triangular, make_identity
    from contextlib import ExitStack

    f32 = mybir.dt.float32
    bf16 = mybir.dt.bfloat16
    AF = mybir.ActivationFunctionType
    Alu = mybir.AluOpType

    nc = bacc.Bacc(None, target_bir_lowering=False, debug=False, num_devices=NC_)

    din = {}
    din["h0"] = nc.dram_tensor("h0", [P, KD, T], bf16, kind="ExternalInput")
    din["wqk"] = nc.dram_tensor("wqk", [L, KD, P, KD, P], bf16, kind="ExternalInput")
    din["bqk"] = nc.dram_tensor("bqk", [L, P, KD], f32, kind="ExternalInput")
    din["wv"] = nc.dram_tensor("wv", [L, P, KD, DQK], bf16, kind="ExternalInput")
    din["wo"] = nc.dram_tensor("wo", [L, KD, P, 3, P], bf16, kind="ExternalInput")
    din["bo"] = nc.dram_tensor("bo", [L, P, KD], f32, kind="ExternalInput")
    din["wup"] = nc.dram_tensor("wup", [L, KFF, P, KD, P], bf16, kind="ExternalInput")
    din["bup"] = nc.dram_tensor("bup", [L, P, KFF], f32, kind="ExternalInput")
    din["wdn"] = nc.dram_tensor("wdn", [L, KD, P, KFF, P], bf16, kind="ExternalInput")
    din["temb"] = nc.dram_tensor("temb", [VN, P, KD, VBLK], bf16, kind="ExternalInput")
    dout = nc.dram_tensor("logits", [T, VSH], bf16, kind="ExternalOutput")

    groups = [[2 * i, 2 * i + 1] for i in range(NC_ // TP)]

    with tile.TileContext(nc) as tc:
        pers = ExitStack()

        const = pers.enter_context(tc.tile_pool(name="const", bufs=1))
        ones_b = const.tile([P, HW], bf16)
        nc.vector.memset(ones_b[:], 1.0)
        maskT = const.tile([P, P], bf16)
        make_upper_triangular(nc, maskT[:], val=MASKVAL, diag=False)
        ident = const.tile([P, P], bf16)
        make_identity(nc, ident[:])
        eps_t = const.tile([1, 1], f32)
        nc.vector.memset(eps_t[:], EPS)
        ones_f = const.tile([1, 1], f32)
        nc.vector.memset(ones_f[:], 1.0)


        hp = pers.enter_context(tc.tile_pool(name="hp", bufs=1))
        h = hp.tile([P, KD * T], bf16)          # resident value is h/2
        xp = pers.enter_context(tc.tile_pool(name="xp", bufs=1))
        xln = xp.tile([P, KD * T], bf16)
        qkp = pers.enter_context(tc.tile_pool(name="qkp", bufs=1))
        qk = qkp.tile([P, KD * T], bf16)
        vtp = pers.enter_context(tc.tile_pool(name="vtp", bufs=1))
        vT = vtp.tile([P, NT * NH * VW], bf16)
        yp = pers.enter_context(tc.tile_pool(name="yp", bufs=1))
        y = yp.tile([P, 3 * T], bf16)           # head pairs packed on 128 parts
        gp = pers.enter_context(tc.tile_pool(name="gp", bufs=1))
        g = gp.tile([P, KFF * T], bf16)
        abcp = pers.enter_context(tc.tile_pool(name="abcp", bufs=1))
        abc = abcp.tile([P, 2 * T], bf16)

        ptp = pers.enter_context(tc.tile_pool(name="ptp", bufs=6))
        sqp = pers.enter_context(tc.tile_pool(name="sqp", bufs=2))
        rbcp = pers.enter_context(tc.tile_pool(name="rbcp", bufs=2))
        lnp = pers.enter_context(tc.tile_pool(name="lnp", bufs=1))
        rsp = pers.enter_context(tc.tile_pool(name="rsp", bufs=1))
        ytp = pers.enter_context(tc.tile_pool(name="ytp", bufs=2))
        otp = pers.enter_context(tc.tile_pool(name="otp", bufs=2))
        prtp = pers.enter_context(tc.tile_pool(name="prtp", bufs=4))

        # 4 + 4 PSUM banks: every tile is <= [128, 512] fp32 (one bank)
        ps_sm = pers.enter_context(tc.tile_pool(name="ps_sm", bufs=4, space="PSUM"))
        ps_av = pers.enter_context(tc.tile_pool(name="ps_av", bufs=4, space="PSUM"))

        dram = pers.enter_context(tc.tile_pool(name="dram", bufs=4, space="DRAM"))

        wqkp = pers.enter_context(tc.tile_pool(name="wqkp", bufs=KD))
        wvp = pers.enter_context(tc.tile_pool(name="wvp", bufs=2))
        wop = pers.enter_context(tc.tile_pool(name="wop", bufs=KD))
        wupp = pers.enter_context(tc.tile_pool(name="wupp", bufs=KFF))
        wdnp = pers.enter_context(tc.tile_pool(name="wdnp", bufs=KD))
        tep = pers.enter_context(tc.tile_pool(name="tep", bufs=4))
        osp = pers.enter_context(tc.tile_pool(name="osp", bufs=4))
        bp = pers.enter_context(tc.tile_pool(name="bp", bufs=3))

        nc.sync.dma_start(out=h[:].rearrange("p (k t) -> p k t", k=KD),
                          in_=din["h0"][:])
        # ones column (index 64) of every per-head V block, for denominator
        vT_4d = vT[:].rearrange("p (t h c) -> p t h c", t=NT, h=NH)
        nc.vector.memset(vT_4d[:, :, :, 64:65], 1.0)

        h3 = h[:].rearrange("p (k t) -> p k t", k=KD)

        def layernorm_half(src, dst, x):
            """Per-token LN of 2*src (src holds h/2) for token half x."""
            tsl = slice(x * HW, (x + 1) * HW)
            s1 = ps_av.tile([1, HW], f32, tag="av", name="s1")
            s2 = ps_av.tile([1, HW], f32, tag="av", name="s2")
            for k in range(KD):
                ksl = slice(k * T + x * HW, k * T + (x + 1) * HW)
                sq = sqp.tile([P, HW], bf16, tag="sq")
                nc.vector.tensor_mul(sq[:], src[:, ksl], src[:, ksl])
                nc.tensor.matmul(s1[0:1, :], ones_b[:, 0:1], src[:, ksl],
                                 start=(k == 0), stop=(k == KD - 1),
                                 skip_group_check=True)
                nc.tensor.matmul(s2[0:1, :], ones_b[:, 0:1], sq[:],
                                 start=(k == 0), stop=(k == KD - 1),
                                 skip_group_check=True)
            lnt = lnp.tile([1, 3 * HW], f32, tag="lnt")
            lnb = lnp.tile([1, 2 * HW], bf16, tag="lnb")
            c0 = lnt[0:1, 0:HW]                  # mean
            c1 = lnt[0:1, HW:2 * HW]             # var -> std
            aa = lnt[0:1, 2 * HW:3 * HW]         # mean^2 -> rstd
            c0b = lnb[0:1, 0:HW]                 # -mean * rstd (bf16)
            aab = lnb[0:1, HW:2 * HW]            # 2 * rstd (bf16)
            nc.vector.tensor_scalar_mul(c0, s1[0:1, :], 2.0 / D)
            nc.vector.tensor_mul(aa, c0, c0)
            nc.vector.scalar_tensor_tensor(c1, s2[0:1, :], 4.0 / D, aa,
                                           op0=Alu.mult, op1=Alu.subtract)
            nc.tensor.matmul(s1[0:1, :], ones_f[0:1, 0:1], aa,
                             start=True, stop=True, skip_group_check=True)
            nc.scalar.activation(c1, c1, AF.Sqrt, bias=eps_t[0:1, 0:1])
            nc.tensor.matmul(s2[0:1, :], ones_f[0:1, 0:1], c1,
                             start=True, stop=True, skip_group_check=True)
            nc.vector.reciprocal_approx_fast(aa, c1)
            nc.vector.scalar_tensor_tensor(c0b, c0, -1.0, aa,
                                           op0=Alu.mult, op1=Alu.mult)
            nc.vector.tensor_scalar_mul(aab, aa, 2.0)
            # HAM tickles: tiny no-output matmuls gated on chain values keep
            # the PE activity window non-idle so the clock stays at 2.4 GHz
            nc.tensor.matmul(s1[0:1, :], ones_b[0:1, 0:1], c0b,
                             start=True, stop=True, skip_group_check=True)
            nc.tensor.matmul(s2[0:1, :], ones_b[0:1, 0:1], aab,
                             start=True, stop=True, skip_group_check=True)
            pa = ps_sm.tile([P, HW], f32, tag="ps")
            nc.tensor.matmul(pa[:], ones_b[0:1, 0:P], aab,
                             start=True, stop=True)
            nc.vector.tensor_copy(abc[:, tsl], pa[:])
            pc = ps_sm.tile([P, HW], f32, tag="ps")
            nc.tensor.matmul(pc[:], ones_b[0:1, 0:P], c0b,
                             start=True, stop=True)
            nc.vector.tensor_copy(abc[:, T + x * HW: T + (x + 1) * HW], pc[:])
            for k in range(KD):
                ksl = slice(k * T + x * HW, k * T + (x + 1) * HW)
                nc.vector.tensor_mul(dst[:, ksl], src[:, ksl], abc[:, tsl])
                nc.vector.tensor_add(dst[:, ksl], dst[:, ksl],
                                     abc[:, T + x * HW: T + (x + 1) * HW])

        def attn_phase(l, x, wqk_t, wv_t, wo_t, bqk_t, bo_t, arm_prev,
                       pre_cb=None, pair_cbs=None):
            """LN1 + qkv + attention + out_proj for token half x.
            pre_cb fires after the qkv GEMMs; pair_cbs[hpair] fires after
            that pair's PV accumulation (before its normalize) -- used to
            interleave the next MLP half's dense up-GEMM chunks into the
            scalar-paced attention stream so the PE clock stays ramped."""
            if arm_prev is not None:
                nc.sync.dma_start(out=h3[:, :, x * HW:(x + 1) * HW],
                                  in_=arm_prev[:, :, :])
            layernorm_half(h, xln, x)
            # qk GEMM
            for m in range(KD):
                ps = ps_sm.tile([P, HW], f32, tag="ps")
                for k in range(KD):
                    nc.tensor.matmul(
                        ps[:], wqk_t[m][:, k * P:(k + 1) * P],
                        xln[:, k * T + x * HW: k * T + (x + 1) * HW],
                        start=(k == 0), stop=(k == KD - 1))
                nc.scalar.activation(
                    qk[:, m * T + x * HW: m * T + (x + 1) * HW],
                    ps[:], AF.Identity, bias=bqk_t[:, m:m + 1])
            # v GEMM (x-stationary): vT[t, 65h+dv], col 64 = ones
            for tb in range(4 * x, 4 * x + 4):
                ps = ps_sm.tile([P, HW], f32, tag="ps")
                for k in range(KD):
                    nc.tensor.matmul(
                        ps[:, 0:DQK], xln[:, k * T + tb * P: k * T + (tb + 1) * P],
                        wv_t[:, k * DQK:(k + 1) * DQK],
                        start=(k == 0), stop=(k == KD - 1))
                nc.any.tensor_copy(vT_4d[:, tb, :, 0:64],
                                   ps[:, 0:DQK].rearrange("p (h c) -> p h c", h=NH))
            if pre_cb is not None:
                pre_cb()
            # attention for query half x: key blocks 0..4*(x+1)
            ktmax = 4 * (x + 1)
            for hpair in range(NH // 2):
                pair = (2 * hpair, 2 * hpair + 1)
                avt = {}
                for hh in pair:
                    avt[hh] = ps_av.tile([VW, HW], f32, tag="av",
                                         name=f"av_{hh}")
                # software-pipelined over kt: PV trails scores by one step
                steps = []   # (kt, hh, a, pt)
                for kt in range(ktmax):
                    diag = (kt // 4) == x
                    a = (kt % 4) * P if diag else 0
                    for hh in pair:
                        po = 64 * (hh % 2)
                        qc = (hh // 2) * T + x * HW
                        kc = (3 + hh // 2) * T
                        st = ps_sm.tile([P, HW], f32, tag="ps")
                        nc.tensor.matmul(
                            st[:, a:HW],
                            qk[po:po + 64, kc + kt * P: kc + (kt + 1) * P],
                            qk[po:po + 64, qc + a: qc + HW],
                            start=True, stop=not diag,
                            skip_group_check=True)
                        if diag:
                            nc.tensor.matmul(st[:, a:a + P], maskT[:], ident[:],
                                             start=False, stop=True,
                                             skip_group_check=True)
                        pt = ptp.tile([P, HW], bf16, tag="pt")
                        nc.scalar.activation(pt[:, a:HW], st[:, a:HW],
                                             AF.Exp, scale=1.0 / np.sqrt(HD))
                        steps.append((kt, hh, a, pt))
                    # issue PV for the previous kt (its exps are done by now)
                    while len(steps) > 2:
                        pkt, phh, pa_, ppt = steps.pop(0)
                        nc.tensor.matmul(
                            avt[phh][0:VW, pa_:HW],
                            vT_4d[:, pkt, phh, :],
                            ppt[:, pa_:HW],
                            start=(pkt == 0),
                            stop=(pkt == ktmax - 1),
                            skip_group_check=True)
                while steps:
                    pkt, phh, pa_, ppt = steps.pop(0)
                    nc.tensor.matmul(
                        avt[phh][0:VW, pa_:HW],
                        vT_4d[:, pkt, phh, :],
                        ppt[:, pa_:HW],
                        start=(pkt == 0),
                        stop=(pkt == ktmax - 1),
                        skip_group_check=True)
                if pair_cbs is not None and hpair in pair_cbs:
                    pair_cbs[hpair]()
                # normalize: one K=2 broadcast matmul per pair; odd head is
                # relocated to partitions 64:128 via SBUF->SBUF DMA
                av0, av1 = avt[pair[0]], avt[pair[1]]
                rsf = rsp.tile([1, 2 * HW], f32, tag="rsf")
                nc.scalar.activation(rsf[0:1, 0:HW], av0[64:65, 0:HW],
                                     AF.Identity)
                nc.scalar.activation(rsf[0:1, HW:2 * HW], av1[64:65, 0:HW],
                                     AF.Identity)
                rs = rsp.tile([1, 2 * HW], f32, tag="rs")
                nc.vector.reciprocal_approx_fast(rs[:], rsf[:])
                rsb = rsp.tile([1, 2 * HW], bf16, tag="rsb")
                nc.vector.tensor_copy(rsb[:], rs[:])
                rb = ps_av.tile([P, HW], f32, tag="av", name="rb")
                nc.tensor.matmul(rb[0:64, :], ones_b[0:1, 0:64],
                                 rsb[0:1, 0:HW], start=True, stop=True,
                                 skip_group_check=True)
                nc.tensor.matmul(rb[64:128, :], ones_b[0:1, 0:64],
                                 rsb[0:1, HW:2 * HW], start=True, stop=True,
                                 skip_group_check=True)
                rbc = rbcp.tile([P, HW], f32, tag="rbc")
                nc.vector.tensor_copy(rbc[:], rb[:])
                ycol = hpair * T + x * HW
                nc.vector.tensor_mul(y[0:64, ycol:ycol + HW],
                                     av0[0:64, :], rbc[0:64, :])
                yt = ytp.tile([64, HW], bf16, tag="yt")
                nc.vector.tensor_copy(yt[:], av1[0:64, :])
                nc.sync.dma_start(out=y[64:128, ycol:ycol + HW], in_=yt[:])
                # HAM tickle into the now-dead rb bank, fires once yt lands
                nc.tensor.matmul(rb[0:1, :], ones_b[0:64, 0:1], yt[:],
                                 start=True, stop=True, skip_group_check=True)
                nc.vector.tensor_mul(y[64:128, ycol:ycol + HW],
                                     y[64:128, ycol:ycol + HW], rbc[64:128, :])
            # out_proj (K=128 over packed head pairs), hp-outer so pair-0/1
            # matmuls run while the last pair's normalize chain completes
            ar_in = dram.tile([P, KD, HW], bf16, tag="ara_in")
            ar_out = dram.tile([P, KD, HW], bf16, tag="ara_out")
            for mg in (0, 3):
                pss = [ps_sm.tile([P, HW], f32, tag="ps", name=f"pso_{i}")
                       for i in range(3)]
                for hp_ in range(3):
                    for i in range(3):
                        nc.tensor.matmul(
                            pss[i][:], wo_t[mg + i][:, hp_ * P:(hp_ + 1) * P],
                            y[:, hp_ * T + x * HW: hp_ * T + (x + 1) * HW],
                            start=(hp_ == 0), stop=(hp_ == 2))
                for i in range(3):
                    m = mg + i
                    ot = otp.tile([P, HW], bf16, tag="ot")
                    nc.scalar.activation(ot[:], pss[i][:], AF.Identity,
                                         bias=bo_t[:, m:m + 1])
                    dst = prtp.tile([P, HW], bf16, tag="prt")
                    nc.vector.scalar_tensor_tensor(
                        dst[:], h[:, m * T + x * HW: m * T + (x + 1) * HW],
                        0.5, ot[:], op0=Alu.mult, op1=Alu.add)
                    nc.sync.dma_start(out=ar_in[:, m, :], in_=dst[:])
            nc.gpsimd.collective_compute(
                "AllReduce", Alu.add, replica_groups=groups,
                ins=[ar_in.opt()], outs=[ar_out.opt()])
            return ar_out

        def up_chunks(x, ms, wup_t, bup_t):
            for m in ms:
                ps = ps_sm.tile([P, HW], f32, tag="ps", name="ps_up")
                for k in range(KD):
                    nc.tensor.matmul(
                        ps[:], wup_t[m][:, k * P:(k + 1) * P],
                        xln[:, k * T + x * HW: k * T + (x + 1) * HW],
                        start=(k == 0), stop=(k == KD - 1))
                nc.scalar.activation(
                    g[:, m * T + x * HW: m * T + (x + 1) * HW],
                    ps[:], AF.Gelu, bias=bup_t[:, m:m + 1])

        def down_phase(l, x, wdn_t):
            """down GEMM + AR for half x (its up chunks ran earlier)."""
            ar_in = dram.tile([P, KD, HW], bf16, tag="arm_in")
            ar_out = dram.tile([P, KD, HW], bf16, tag="arm_out")
            for m in range(KD):
                ps = ps_sm.tile([P, HW], f32, tag="ps")
                for k in range(KFF):
                    nc.tensor.matmul(
                        ps[:], wdn_t[m][:, k * P:(k + 1) * P],
                        g[:, k * T + x * HW: k * T + (x + 1) * HW],
                        start=(k == 0), stop=(k == KFF - 1))
                dst = prtp.tile([P, HW], bf16, tag="prt")
                nc.vector.scalar_tensor_tensor(
                    dst[:], h[:, m * T + x * HW: m * T + (x + 1) * HW],
                    0.5, ps[:], op0=Alu.mult, op1=Alu.add)
                nc.sync.dma_start(out=ar_in[:, m, :], in_=dst[:])
            nc.gpsimd.collective_compute(
                "AllReduce", Alu.add, replica_groups=groups,
                ins=[ar_in.opt()], outs=[ar_out.opt()])
            return ar_out

        def mlp_phase(l, x, wup_t, wdn_t, bup_t, ara):
            """h update from attention AR, LN2 + MLP for half x."""
            nc.sync.dma_start(out=h3[:, :, x * HW:(x + 1) * HW], in_=ara[:, :, :])
            layernorm_half(h, xln, x)
            for m in range(KFF):
                ps = ps_sm.tile([P, HW], f32, tag="ps")
                for k in range(KD):
                    nc.tensor.matmul(
                        ps[:], wup_t[m][:, k * P:(k + 1) * P],
                        xln[:, k * T + x * HW: k * T + (x + 1) * HW],
                        start=(k == 0), stop=(k == KD - 1))
                nc.scalar.activation(
                    g[:, m * T + x * HW: m * T + (x + 1) * HW],
                    ps[:], AF.Gelu, bias=bup_t[:, m:m + 1])
            ar_in = dram.tile([P, KD, HW], bf16, tag="arm_in")
            ar_out = dram.tile([P, KD, HW], bf16, tag="arm_out")
            for m in range(KD):
                ps = ps_sm.tile([P, HW], f32, tag="ps")
                for k in range(KFF):
                    nc.tensor.matmul(
                        ps[:], wdn_t[m][:, k * P:(k + 1) * P],
                        g[:, k * T + x * HW: k * T + (x + 1) * HW],
                        start=(k == 0), stop=(k == KFF - 1))
                dst = prtp.tile([P, HW], bf16, tag="prt")
                nc.vector.scalar_tensor_tensor(
                    dst[:], h[:, m * T + x * HW: m * T + (x + 1) * HW],
                    0.5, ps[:], op0=Alu.mult, op1=Alu.add)
                nc.sync.dma_start(out=ar_in[:, m, :], in_=dst[:])
            nc.gpsimd.collective_compute(
                "AllReduce", Alu.add, replica_groups=groups,
                ins=[ar_in.opt()], outs=[ar_out.opt()])
            return ar_out

        arm = [None, None]
        for l in range(L):
            # load the whole layer's weights up front (used by both halves)
            wqk_t = []
            for m in range(KD):
                wt = wqkp.tile([P, KD * P], bf16, tag="wqk")
                nc.sync.dma_start(
                    out=wt[:].rearrange("p (k c) -> p k c", k=KD),
                    in_=din["wqk"][l, m])
                wqk_t.append(wt)
            wv_t = wvp.tile([P, KD * DQK], bf16, tag="wv")
            nc.sync.dma_start(
                out=wv_t[:].rearrange("p (k c) -> p k c", k=KD),
                in_=din["wv"][l])
            wo_t = []
            for m in range(KD):
                wt = wop.tile([P, 3 * P], bf16, tag="wo")
                nc.sync.dma_start(
                    out=wt[:].rearrange("p (k c) -> p k c", k=3),
                    in_=din["wo"][l, m])
                wo_t.append(wt)
            wup_t = []
            for m in range(KFF):
                wt = wupp.tile([P, KD * P], bf16, tag="wup")
                nc.sync.dma_start(
                    out=wt[:].rearrange("p (k c) -> p k c", k=KD),
                    in_=din["wup"][l, m])
                wup_t.append(wt)
            wdn_t = []
            for m in range(KD):
                wt = wdnp.tile([P, KFF * P], bf16, tag="wdn")
                nc.sync.dma_start(
                    out=wt[:].rearrange("p (k c) -> p k c", k=KFF),
                    in_=din["wdn"][l, m])
                wdn_t.append(wt)
            bqk_t = bp.tile([P, KD], f32, tag="bqk")
            nc.sync.dma_start(out=bqk_t[:], in_=din["bqk"][l])
            bo_t = bp.tile([P, KD], f32, tag="bo")
            nc.sync.dma_start(out=bo_t[:], in_=din["bo"][l])
            bup_t = bp.tile([P, KFF], f32, tag="bup")
            nc.sync.dma_start(out=bup_t[:], in_=din["bup"][l])

            ara0 = attn_phase(l, 0, wqk_t, wv_t, wo_t, bqk_t, bo_t, arm[0])

            def ln2_h0():
                nc.sync.dma_start(out=h3[:, :, 0:HW], in_=ara0[:, :, :])
                layernorm_half(h, xln, 0)

            ara1 = attn_phase(
                l, 1, wqk_t, wv_t, wo_t, bqk_t, bo_t, arm[1],
                pre_cb=ln2_h0,
                pair_cbs={
                    0: lambda: up_chunks(0, range(0, 4), wup_t, bup_t),
                    1: lambda: up_chunks(0, range(4, 8), wup_t, bup_t),
                })
            up_chunks(0, range(8, KFF), wup_t, bup_t)
            arm[0] = down_phase(l, 0, wdn_t)
            arm[1] = mlp_phase(l, 1, wup_t, wdn_t, bup_t, ara1)

        # ---- final LN (lnf folded into temb on host) + lm_head ----
        # pipelined so the last AR (half 1) hides behind half-0 logits work
        nc.sync.dma_start(out=h3[:, :, 0:HW], in_=arm[0][:, :, :])
        layernorm_half(h, xln, 0)

        def lm_block(n, te, m):
            ps = ps_sm.tile([P, HW], f32, tag="ps")
            for k in range(KD):
                nc.tensor.matmul(
                    ps[:, 0:VBLK], xln[:, k * T + m * P: k * T + (m + 1) * P],
                    te[:, k * VBLK:(k + 1) * VBLK],
                    start=(k == 0), stop=(k == KD - 1))
            ot = osp.tile([P, VBLK], bf16, tag="lmot")
            nc.any.tensor_copy(ot[:], ps[:, 0:VBLK])
            nc.sync.dma_start(
                out=dout[m * P:(m + 1) * P, n * VBLK:(n + 1) * VBLK],
                in_=ot[:])

        tes = []
        for n in range(4):
            te = tep.tile([P, KD * VBLK], bf16, tag="te")
            nc.sync.dma_start(
                out=te[:].rearrange("p (k c) -> p k c", k=KD),
                in_=din["temb"][n])
            tes.append(te)
            for m in range(4):
                lm_block(n, te, m)
        nc.sync.dma_start(out=h3[:, :, HW:T], in_=arm[1][:, :, :])
        layernorm_half(h, xln, 1)
        for n in range(4):
            for m in range(4, NT):
                lm_block(n, tes[n], m)
        for n in range(4, VN):
            te = tep.tile([P, KD * VBLK], bf16, tag="te")
            nc.sync.dma_start(
                out=te[:].rearrange("p (k c) -> p k c", k=KD),
                in_=din["temb"][n])
            for m in range(NT):
                lm_block(n, te, m)
        pers.close()

    nc.compile()
    return nc


# --------------------------------------------------------------------------
# entry point
# --------------------------------------------------------------------------

def kernel(**inputs):
    import time
    t0 = time.time()
    in_maps = prep_all_inputs(**inputs)
    _CACHE["t_prep"] = time.time() - t0

    if "nc" not in _CACHE:
        t0 = time.time()
        _CACHE["nc"] = build_program()
        _CACHE["t_build"] = time.time() - t0
    nc = _CACHE["nc"]

    from concourse.bass_utils import run_bass_kernel_spmd
    t0 = time.time()
    want_trace = bool(int(os.environ.get("GPT_TRACE", "0")))
    try:
        res = run_bass_kernel_spmd(nc, in_maps, core_ids=list(range(NC_)),
                                   trace=want_trace)
    except ModuleNotFoundError:
        res = run_bass_kernel_spmd(nc, in_maps, core_ids=list(range(NC_)),
                                   trace=False)
    _CACHE["t_run"] = time.time() - t0
    _CACHE["last_result"] = res

    logits = np.empty((B, T, V), np.float32)
    for c in range(NC_):
        b, r = c // TP, c % TP
        logits[b, :, r * VSH:(r + 1) * VSH] = np.asarray(
            res.results[c]["logits"], dtype=np.float32)

    lnf_b = np.asarray(inputs["lnf_b"], np.float32)
    if np.any(lnf_b):
        corr = np.asarray(inputs["tok_emb"], np.float32) @ lnf_b
        logits += corr[None, None, :]
    return logits
